# revision 35
# baseline (speedup 1.0000x reference)
"""InternLM3 custom attention on 8 TRN2 NeuronCores.

Sharding: heads 4-per-core for K/V projection + attention (qk_w/v_w
column-parallel by head); AllToAll converts the attention output from
head-sharded to sequence-sharded; o-projection runs sequence-parallel
(full o_w per core) so each core emits a [256, 2048] output slice.

v2: bf16 streaming path (fp32 PSUM accumulation), software-pipelined
projection -> RoPE -> attention per 512-sequence chunk so the exp
(Activation engine) overlaps the projection matmuls (PE), big resident
SBUF tiles loaded with few large DMAs split across both HWDGE queues
(SP + Activation), o_w prefetched during attention. X-RoPE is folded
into host prep. Attention is computed transposed (S^T[k, q]) so softmax
probabilities feed the PV matmul directly; the softmax denominator
rides along as a ones column appended to V, and its broadcast
reciprocal shares the attention PSUM bank (partitions 64..127).
Causality: strictly-upper k-blocks skipped; diagonal blocks compute
exp/PV only on columns >= r with one [128,128] triangular mask.
"""

import sys

sys.path.insert(0, "/opt/trn_rl_repo")

import numpy as np
import ml_dtypes

import concourse.bass as bass
import concourse.tile as tile
from concourse import bacc, mybir
from concourse.bass import ds, ts
from concourse.bass_utils import run_bass_kernel_spmd

F32 = mybir.dt.float32
BF16 = mybir.dt.bfloat16
NCORES = 8
S = 2048          # sequence
HID = 2048        # hidden
NH = 32           # total heads
HD = 64           # head dim
HPC = NH // NCORES      # heads per core = 4
DPC = HPC * HD          # head-dims per core = 256
SSL = S // NCORES       # output seq slice per core = 256
VW = 66                 # interleaved V stride: 64 dims + 1 ones + 1 pad
ROPE_THETA = 10000.0

# packed input blob layout (bf16 elements, per core)
_BLOB_SPEC = [
    ("hidT", HID * S),
    ("qkwT", HID * DPC),
    ("vwT", HID * DPC),
    ("owT", HID * HID),
    ("xT", DPC * S),
    ("cosT", 128 * S),
    ("sinT", 128 * S),
    ("triT", 128 * 128),
    ("permT", 128 * 128),
]
BLOB_OFFS = {}
_off = 0
for _nm, _n in _BLOB_SPEC:
    BLOB_OFFS[_nm] = (_off, _n)
    _off += _n
BLOB_ELEMS = _off


def build_program(collective=True, dbg=False):
    nc = bacc.Bacc("TRN2", target_bir_lowering=False, debug=False,
                   num_devices=NCORES)

    # ---- I/O: one packed bf16 input blob (fewer per-iteration buffer
    # ---- handles on the dispatch path); fp32 out ----
    blob = nc.dram_tensor("blob", [BLOB_ELEMS], BF16, kind="ExternalInput").ap()

    def bslice(name):
        off, n = BLOB_OFFS[name]
        return blob[ds(off, n)]

    hidT = bslice("hidT").rearrange("(n p s) -> p n s", p=128, s=S)
    qkwT = bslice("qkwT").rearrange("(n p d) -> p n d", p=128, d=DPC)
    vwT = bslice("vwT").rearrange("(n p d) -> p n d", p=128, d=DPC)
    owT = bslice("owT").rearrange("(n p d) -> p n d", p=128, d=HID)
    xT_in = bslice("xT").rearrange("(t p s) -> p t s", p=128, s=S)
    cosT = bslice("cosT").rearrange("(p s) -> p s", p=128)
    sinT = bslice("sinT").rearrange("(p s) -> p s", p=128)
    triT = bslice("triT").rearrange("(p q) -> p q", p=128)
    permT = bslice("permT").rearrange("(p q) -> p q", p=128)
    out_sl = nc.dram_tensor("out_slice", [SSL, HID], F32,
                            kind="ExternalOutput").ap()
    if dbg:
        kt_out = nc.dram_tensor("kt_out", [128, 2 * S], BF16,
                                kind="ExternalOutput").ap()
        vt_out = nc.dram_tensor("vt_out", [128, 16 * VW * HPC], BF16,
                                kind="ExternalOutput").ap()
        att_out = nc.dram_tensor("att_out", [128, 2 * S], BF16,
                                 kind="ExternalOutput").ap()
        afu_out = nc.dram_tensor("afu_out", [128, 16 * SSL], BF16,
                                 kind="ExternalOutput").ap()

    with tile.TileContext(nc) as tc:
        with (
            nc.allow_low_precision(reason="bf16 streaming, fp32 psum accum"),
            tc.tile_pool(name="const", bufs=1) as const,
            tc.tile_pool(name="dram", bufs=1, space="DRAM") as dram,
        ):
            # ---- persistent SBUF residents ----
            qkw_t = const.tile([128, 16, DPC], BF16)
            vw_t = const.tile([128, 16, DPC], BF16)
            xt = const.tile([128, 2, S], BF16)      # X^T, rope'd on host
            cos_t = const.tile([128, S], BF16)
            sin_t = const.tile([128, S], BF16)
            tri_t = const.tile([128, 128], BF16)
            perm_t = const.tile([128, 128], BF16)
            ow_t = const.tile([128, 16, HID], BF16)
            kt = const.tile([128, 2, S], BF16)      # K^T, rope'd in place
            v_t = const.tile([128, 16, VW * HPC], BF16)
            att_t = const.tile([128, 2, S], BF16)   # attn^T assembled
            ones_t = const.tile([1, HD], BF16)

            nc.vector.memset(ones_t[:], 1.0)
            # ones column of V (denominator accumulator)
            nc.vector.memset(
                v_t[:].rearrange("p st (h w) -> p st h w", w=VW)[:, :, :, HD:HD + 1],
                1.0)

            # ===== pipelined: per 512-seq chunk, project K/V, rope K,
            # ===== then attention q-block j=sq (needs K/V chunks <= sq).
            with (
                tc.tile_pool(name="hidp", bufs=1) as hidp,
                tc.tile_pool(name="psk", bufs=2, space="PSUM") as psk,
                tc.tile_pool(name="psv", bufs=2, space="PSUM") as psv,
                tc.tile_pool(name="pss", bufs=2, space="PSUM") as pss,
                tc.tile_pool(name="pspv", bufs=2, space="PSUM") as pspv,
                tc.tile_pool(name="sw", bufs=2) as swp,
                tc.tile_pool(name="pp", bufs=4) as ppool,
                tc.tile_pool(name="rr", bufs=2) as rrp,
            ):
                hid_t = hidp.tile([128, 16, S], BF16)
                # SP queue: split first loads so K matmuls start early;
                # vw only needed once the K half of chunk 0 is done.
                nc.sync.dma_start(out=qkw_t[:, 0:2, :], in_=qkwT[:, 0:2, :])
                nc.sync.dma_start(out=hid_t[:, 0:2, ts(0, 512)],
                                  in_=hidT[:, 0:2, ts(0, 512)])
                nc.sync.dma_start(out=qkw_t[:, 2:8, :], in_=qkwT[:, 2:8, :])
                nc.sync.dma_start(out=hid_t[:, 2:8, ts(0, 512)],
                                  in_=hidT[:, 2:8, ts(0, 512)])
                nc.sync.dma_start(out=qkw_t[:, 8:16, :], in_=qkwT[:, 8:16, :])
                nc.sync.dma_start(out=hid_t[:, 8:16, ts(0, 512)],
                                  in_=hidT[:, 8:16, ts(0, 512)])
                nc.sync.dma_start(out=vw_t[:], in_=vwT)
                for sq in range(1, 4):
                    nc.sync.dma_start(out=hid_t[:, :, ts(sq, 512)],
                                      in_=hidT[:, :, ts(sq, 512)])
                # o_w prefetch: after the phase A loads so it doesn't
                # steal DMA bandwidth from them; lands well before o-proj.
                nc.sync.dma_start(out=ow_t[:], in_=owT)
                # ACT queue: rope/attention consts (small, needed early).
                nc.scalar.dma_start(out=cos_t[:], in_=cosT)
                nc.scalar.dma_start(out=sin_t[:], in_=sinT)
                nc.scalar.dma_start(out=xt[:], in_=xT_in)
                nc.scalar.dma_start(out=tri_t[:], in_=triT)
                nc.scalar.dma_start(out=perm_t[:], in_=permT)
                for sq in range(4):
                    sqs = ds(512 * sq, 512)
                    # ---- phase A chunk: K^T then V for seq block sq ----
                    pk = [psk.tile([128, 512], F32, tag='pk', name='pk')
                          for _ in range(2)]
                    for hc in range(16):
                        for m in range(2):
                            nc.tensor.matmul(
                                pk[m][:],
                                (qkw_t[:, hc, ts(m, 128)]),
                                (hid_t[:, hc, sqs]),
                                start=(hc == 0), stop=(hc == 15))
                    # K: copy to bf16, rotate-half via PE permutation matmul,
                    # rope in place (sin sign folded on host)
                    ks = swp.tile([128, 2, 512], BF16, tag="sw")
                    for t in range(2):
                        nc.vector.tensor_copy(out=kt[:, t, sqs], in_=pk[t][:])
                    for t in range(2):
                        ksp = pss.tile([128, 512], F32, tag='sp')
                        nc.tensor.matmul(ksp[:], (perm_t[:]),
                                         (kt[:, t, sqs]),
                                         start=True, stop=True)
                        nc.vector.tensor_mul(out=ks[:, t, :], in0=ksp[:],
                                             in1=sin_t[:, sqs])
                        nc.vector.tensor_mul(out=kt[:, t, sqs],
                                             in0=kt[:, t, sqs],
                                             in1=cos_t[:, sqs])
                        nc.vector.tensor_add(out=kt[:, t, sqs],
                                             in0=kt[:, t, sqs],
                                             in1=ks[:, t, :])
                    # V: one psum group per bank slot (start=True zeroes the
                    # whole 2KB zero-region, so groups must not share a bank)
                    for st4 in range(4):
                        pvt = psv.tile([128, 256], F32, tag='pv', name='pv')
                        for hc in range(16):
                            nc.tensor.matmul(
                                pvt[:],
                                (hid_t[:, hc, ds(512 * sq + 128 * st4, 128)]),
                                (vw_t[:, hc, :]),
                                start=(hc == 0), stop=(hc == 15))
                        nc.vector.tensor_copy(
                            out=v_t[:, sq * 4 + st4, :].rearrange(
                                "p (h w) -> p h w", w=VW)[:, :, 0:HD],
                            in_=pvt[:].rearrange("p (h d) -> p h d", d=HD))

                    # ---- phase B: attention q-block j == sq, all 4 heads ----
                    j = sq
                    q0 = 512 * j
                    nk = 4 * (j + 1)
                    for h in range(HPC):
                        hp = HD * (h % 2)
                        htl = h // 2
                        pvp = pspv.tile([HD + 1, 512], F32, tag='pvp')
                        for i in range(nk):
                            r = 128 * i - q0
                            w0 = max(r, 0)
                            sp = pss.tile([128, 512], F32, tag='sp')
                            nc.tensor.matmul(
                                sp[:, ds(w0, 512 - w0)],
                                (kt[hp:hp + HD, htl, ts(i, 128)]),
                                (xt[hp:hp + HD, htl, ds(q0 + w0, 512 - w0)]),
                                start=True, stop=True)
                            pt = ppool.tile([128, 512], BF16, tag="pt")
                            nc.scalar.activation(
                                out=pt[:, ds(w0, 512 - w0)],
                                in_=sp[:, ds(w0, 512 - w0)],
                                func=mybir.ActivationFunctionType.Exp,
                                scale=0.125)
                            if r >= 0:   # diagonal: ragged triangle mask
                                nc.vector.tensor_mul(
                                    out=pt[:, ds(r, 128)], in0=pt[:, ds(r, 128)],
                                    in1=tri_t[:])
                            nc.tensor.matmul(
                                pvp[0:HD + 1, ds(w0, 512 - w0)],
                                (v_t[:, i, ds(VW * h, HD + 1)]),
                                (pt[:, ds(w0, 512 - w0)]),
                                start=(i == 0), stop=(i == nk - 1))
                        # denominator: reciprocal row, broadcast into the
                        # unused partitions 64..127 of the same psum bank
                        rec = rrp.tile([1, 512], BF16, tag="rec")
                        nc.vector.reciprocal(out=rec[:], in_=pvp[HD:HD + 1, :])
                        bc = pss.tile([HD, 512], F32, tag='sp')
                        nc.tensor.matmul(bc[:], (ones_t[:]),
                                         (rec[:]), start=True, stop=True)
                        bcs = rrp.tile([HD, 512], BF16, tag="bcs")
                        nc.vector.tensor_copy(out=bcs[:], in_=bc[:])
                        nc.vector.tensor_mul(
                            out=att_t[hp:hp + HD, htl, ds(q0, 512)],
                            in0=pvp[0:HD, :],
                            in1=bcs[:])

            if dbg:
                nc.sync.dma_start(out=kt_out[:],
                                  in_=kt[:].rearrange("p t s -> p (t s)"))
                nc.sync.dma_start(out=vt_out[:],
                                  in_=v_t[:].rearrange("p a b -> p (a b)"))
                nc.sync.dma_start(out=att_out[:],
                                  in_=att_t[:].rearrange("p t s -> p (t s)"))

            # =========== AllToAll: head-sharded -> seq-sharded ===========
            # Split by head-pair tile t: the t=0 collective fires as soon as
            # heads 0-1 finish, overlapping the t=1 attention tail and the
            # even-hc half of the o-projection with the t=1 collective.
            a2a_in = [dram.tile([NCORES, 128, SSL], BF16, name=f"a2ain{t}")
                      for t in range(2)]
            a2a_out = [dram.tile([NCORES * 128, SSL], BF16, name=f"a2aout{t}")
                       for t in range(2)]
            for t in range(2):
                nc.sync.dma_start(
                    out=a2a_in[t][:].rearrange("d p s -> p d s"),
                    in_=att_t[:, t, :].rearrange("p (d s) -> p d s", d=NCORES))
                if collective:
                    nc.gpsimd.collective_compute(
                        "AllToAll",
                        mybir.AluOpType.bypass,
                        replica_groups=[list(range(NCORES))],
                        ins=[a2a_in[t][:].opt()],
                        outs=[a2a_out[t][:].opt()],
                    )
                else:
                    # timeline-sim mock: same-size DRAM->DRAM move
                    nc.sync.dma_start(
                        out=a2a_out[t][:],
                        in_=a2a_in[t][:].rearrange("d p s -> (d p) s"))

            # =========== o-projection (sequence-parallel) ===========
            with (
                tc.tile_pool(name="af", bufs=1) as afp,
                tc.tile_pool(name="ob", bufs=1) as obp,
                tc.tile_pool(name="pso", bufs=8, space="PSUM") as pso,
            ):
                afull = afp.tile([128, 16, SSL], BF16)
                for t in range(2):
                    # a2a_out[t] rows (d p) hold global attn dims 256d+128t+p
                    # -> afull n slices t, t+2, t+4, ...
                    nc.sync.dma_start(
                        out=afull[:].rearrange(
                            "p (d u) s -> p d u s", u=2)[:, :, t, :],
                        in_=a2a_out[t][:].rearrange("(d p) s -> p d s", p=128))
                if dbg:
                    nc.sync.dma_start(out=afu_out[:],
                                      in_=afull[:].rearrange("p a b -> p (a b)"))
                osb = obp.tile([128, 2, HID], F32)
                po = [[pso.tile([128, 512], F32, tag='po', name='po')
                       for t in range(2)] for ob in range(4)]
                # even hc chunks depend only on the t=0 collective, odd on
                # t=1: run all even ones first so they overlap collective 1.
                for hc in [2 * i for i in range(8)] + [2 * i + 1 for i in range(8)]:
                    for ob in range(4):
                        for t in range(2):
                            nc.tensor.matmul(
                                po[ob][t][:],
                                (afull[:, hc, ts(t, 128)]),
                                (ow_t[:, hc, ts(ob, 512)]),
                                start=(hc == 0), stop=(hc == 15))
                for ob in range(4):
                    nc.scalar.copy(out=osb[:, 0, ts(ob, 512)],
                                   in_=po[ob][0][:])
                    nc.vector.tensor_copy(out=osb[:, 1, ts(ob, 512)],
                                          in_=po[ob][1][:])
                nc.sync.dma_start(out=out_sl[ts(0, 128), :], in_=osb[:, 0, :])
                nc.scalar.dma_start(out=out_sl[ts(1, 128), :], in_=osb[:, 1, :])

    nc.compile()
    return nc


_PROGRAM = None


def _host_inputs(hidden_states, qk_w, v_w, o_w, position_ids):
    bf16 = ml_dtypes.bfloat16
    hs = np.asarray(hidden_states, dtype=np.float32)[0]          # [S, HID]
    qk_w = np.asarray(qk_w, dtype=np.float32)
    v_w = np.asarray(v_w, dtype=np.float32)
    o_w = np.asarray(o_w, dtype=np.float32)
    pos = np.asarray(position_ids)[0].astype(np.float64)         # [S]

    hidT = np.ascontiguousarray(hs.T)                            # [HID, S]
    hidT_bf = hidT.astype(bf16)
    owT_bf = np.ascontiguousarray(o_w.T).astype(bf16)            # [HID, HID]

    inv_freq = 1.0 / (ROPE_THETA ** (np.arange(0, HD, 2, dtype=np.float64) / HD))
    freqs = pos[None, :] * inv_freq[:, None]                     # [32, S]
    emb = np.concatenate([freqs, freqs], axis=0)                 # [64, S]
    cos1 = np.cos(emb).astype(np.float32)
    sin1 = np.sin(emb).astype(np.float32)
    sin_signed = sin1.copy()
    sin_signed[:HD // 2] *= -1.0                                 # fold rotate sign
    cosT = np.tile(cos1, (2, 1)).astype(bf16)                    # [128, S]
    sinT = np.tile(sin_signed, (2, 1)).astype(bf16)

    kl = np.arange(128)[:, None]
    u = np.arange(128)[None, :]
    triT = (u >= kl).astype(bf16)                                # [128, 128]

    # rotate-half row permutation (symmetric involution, per 64-row head)
    idx = np.arange(128)
    src = (idx // HD) * HD + (idx % HD + HD // 2) % HD
    permT = np.zeros((128, 128), np.float32)
    permT[idx, src] = 1.0
    permT = permT.astype(bf16)

    in_maps = []
    for c in range(NCORES):
        rows = slice(DPC * c, DPC * (c + 1))
        xT = hidT[rows]                                          # [256, S] fp32
        # host-side X RoPE: x*cos + rotate_half(x)*sin per 64-row head
        xTs = np.empty_like(xT)
        for h in range(HPC):
            b = HD * h
            xTs[b:b + 32] = -xT[b + 32:b + HD]
            xTs[b + 32:b + HD] = xT[b:b + 32]
        cs = np.tile(cos1, (HPC, 1))                             # [256, S]
        sn = np.tile(sin1, (HPC, 1))
        xTr = (xT * cs + xTs * sn).astype(bf16)
        parts = {
            "hidT": hidT_bf,
            "qkwT": np.ascontiguousarray(qk_w[rows].T).astype(bf16),
            "vwT": np.ascontiguousarray(v_w[rows].T).astype(bf16),
            "owT": owT_bf,
            "xT": xTr,
            "cosT": cosT,
            "sinT": sinT,
            "triT": triT,
            "permT": permT,
        }
        blob = np.concatenate([parts[nm].ravel() for nm, _ in _BLOB_SPEC])
        assert blob.size == BLOB_ELEMS
        in_maps.append({"blob": blob})
    return in_maps


def kernel(hidden_states, qk_w, v_w, o_w, position_ids, **extra):
    global _PROGRAM
    if _PROGRAM is None:
        _PROGRAM = build_program()
    in_maps = _host_inputs(hidden_states, qk_w, v_w, o_w, position_ids)
    res = run_bass_kernel_spmd(_PROGRAM, in_maps, list(range(NCORES)))
    out = np.concatenate([res.results[c]["out_slice"]
                          for c in range(NCORES)], axis=0)
    return out.reshape(1, S, HID).astype(np.float32)


# revision 37
# speedup vs baseline: 1.2993x; 1.2993x over previous
"""InternLM3 custom attention on 8 TRN2 NeuronCores.

Sharding: heads 4-per-core for K/V projection + attention (qk_w/v_w
column-parallel by head); AllToAll converts the attention output from
head-sharded to sequence-sharded; o-projection runs sequence-parallel
(full o_w per core) so each core emits a [256, 2048] output slice.

v2: bf16 streaming path (fp32 PSUM accumulation), software-pipelined
projection -> RoPE -> attention per 512-sequence chunk so the exp
(Activation engine) overlaps the projection matmuls (PE), big resident
SBUF tiles loaded with few large DMAs split across both HWDGE queues
(SP + Activation), o_w prefetched during attention. X-RoPE is folded
into host prep. Attention is computed transposed (S^T[k, q]) so softmax
probabilities feed the PV matmul directly; the softmax denominator
rides along as a ones column appended to V, and its broadcast
reciprocal shares the attention PSUM bank (partitions 64..127).
Causality: strictly-upper k-blocks skipped; diagonal blocks compute
exp/PV only on columns >= r with one [128,128] triangular mask.
"""

import sys

sys.path.insert(0, "/opt/trn_rl_repo")

import numpy as np
import ml_dtypes

import concourse.bass as bass
import concourse.tile as tile
from concourse import bacc, mybir
from concourse.bass import ds, ts
from concourse.bass_utils import run_bass_kernel_spmd

F32 = mybir.dt.float32
BF16 = mybir.dt.bfloat16
NCORES = 8
S = 2048          # sequence
HID = 2048        # hidden
NH = 32           # total heads
HD = 64           # head dim
HPC = NH // NCORES      # heads per core = 4
DPC = HPC * HD          # head-dims per core = 256
SSL = S // NCORES       # output seq slice per core = 256
VW = 66                 # interleaved V stride: 64 dims + 1 ones + 1 pad
ROPE_THETA = 10000.0

# packed input blob layout (bf16 elements, per core)
_BLOB_SPEC = [
    ("hidT", HID * S),
    ("qkwT", HID * DPC),
    ("vwT", HID * DPC),
    ("owT", HID * HID),
    ("xT", DPC * S),
    ("cosT", 128 * S),
    ("sinT", 128 * S),
    ("triT", 128 * 128),
    ("permT", 128 * 128),
]
BLOB_OFFS = {}
_off = 0
for _nm, _n in _BLOB_SPEC:
    BLOB_OFFS[_nm] = (_off, _n)
    _off += _n
BLOB_ELEMS = _off


def build_program(collective=True, dbg=False):
    nc = bacc.Bacc("TRN2", target_bir_lowering=False, debug=False,
                   num_devices=NCORES)

    # ---- I/O: one packed bf16 input blob (fewer per-iteration buffer
    # ---- handles on the dispatch path); fp32 out ----
    blob = nc.dram_tensor("blob", [BLOB_ELEMS], BF16, kind="ExternalInput").ap()

    def bslice(name):
        off, n = BLOB_OFFS[name]
        return blob[ds(off, n)]

    hidT = bslice("hidT").rearrange("(n p s) -> p n s", p=128, s=S)
    qkwT = bslice("qkwT").rearrange("(n p d) -> p n d", p=128, d=DPC)
    vwT = bslice("vwT").rearrange("(n p d) -> p n d", p=128, d=DPC)
    owT = bslice("owT").rearrange("(n p d) -> p n d", p=128, d=HID)
    xT_in = bslice("xT").rearrange("(t p s) -> p t s", p=128, s=S)
    cosT = bslice("cosT").rearrange("(p s) -> p s", p=128)
    sinT = bslice("sinT").rearrange("(p s) -> p s", p=128)
    triT = bslice("triT").rearrange("(p q) -> p q", p=128)
    permT = bslice("permT").rearrange("(p q) -> p q", p=128)
    out_sl = nc.dram_tensor("out_slice", [SSL, HID], F32,
                            kind="ExternalOutput").ap()
    if dbg:
        kt_out = nc.dram_tensor("kt_out", [128, 2 * S], BF16,
                                kind="ExternalOutput").ap()
        vt_out = nc.dram_tensor("vt_out", [128, 16 * VW * HPC], BF16,
                                kind="ExternalOutput").ap()
        att_out = nc.dram_tensor("att_out", [128, 2 * S], BF16,
                                 kind="ExternalOutput").ap()
        afu_out = nc.dram_tensor("afu_out", [128, 16 * SSL], BF16,
                                 kind="ExternalOutput").ap()

    with tile.TileContext(nc) as tc:
        with (
            nc.allow_low_precision(reason="bf16 streaming, fp32 psum accum"),
            tc.tile_pool(name="const", bufs=1) as const,
            tc.tile_pool(name="dram", bufs=1, space="DRAM") as dram,
        ):
            # ---- persistent SBUF residents ----
            qkw_t = const.tile([128, 16, DPC], BF16)
            vw_t = const.tile([128, 16, DPC], BF16)
            xt = const.tile([128, 2, S], BF16)      # X^T, rope'd on host
            cos_t = const.tile([128, S], BF16)
            sin_t = const.tile([128, S], BF16)
            tri_t = const.tile([128, 128], BF16)
            perm_t = const.tile([128, 128], BF16)
            ow_t = const.tile([128, 16, HID], BF16)
            kt = const.tile([128, 2, S], BF16)      # K^T, rope'd in place
            v_t = const.tile([128, 16, VW * HPC], BF16)
            att_t = const.tile([128, 2, S], BF16)   # attn^T assembled
            ones_t = const.tile([1, HD], BF16)

            nc.vector.memset(ones_t[:], 1.0)
            # ones column of V (denominator accumulator)
            nc.vector.memset(
                v_t[:].rearrange("p st (h w) -> p st h w", w=VW)[:, :, :, HD:HD + 1],
                1.0)

            # ===== pipelined: per 512-seq chunk, project K/V, rope K,
            # ===== then attention q-block j=sq (needs K/V chunks <= sq).
            with (
                tc.tile_pool(name="hidp", bufs=1) as hidp,
                tc.tile_pool(name="psk", bufs=2, space="PSUM") as psk,
                tc.tile_pool(name="psv", bufs=2, space="PSUM") as psv,
                tc.tile_pool(name="pss", bufs=2, space="PSUM") as pss,
                tc.tile_pool(name="pspv", bufs=2, space="PSUM") as pspv,
                tc.tile_pool(name="sw", bufs=2) as swp,
                tc.tile_pool(name="pp", bufs=4) as ppool,
                tc.tile_pool(name="rr", bufs=2) as rrp,
            ):
                hid_t = hidp.tile([128, 16, S], BF16)
                # SP queue: split first loads so K matmuls start early;
                # vw only needed once the K half of chunk 0 is done.
                nc.sync.dma_start(out=qkw_t[:, 0:2, :], in_=qkwT[:, 0:2, :])
                nc.sync.dma_start(out=hid_t[:, 0:2, ts(0, 512)],
                                  in_=hidT[:, 0:2, ts(0, 512)])
                nc.sync.dma_start(out=qkw_t[:, 2:8, :], in_=qkwT[:, 2:8, :])
                nc.sync.dma_start(out=hid_t[:, 2:8, ts(0, 512)],
                                  in_=hidT[:, 2:8, ts(0, 512)])
                nc.sync.dma_start(out=qkw_t[:, 8:16, :], in_=qkwT[:, 8:16, :])
                nc.sync.dma_start(out=hid_t[:, 8:16, ts(0, 512)],
                                  in_=hidT[:, 8:16, ts(0, 512)])
                nc.sync.dma_start(out=vw_t[:], in_=vwT)
                for sq in range(1, 4):
                    nc.sync.dma_start(out=hid_t[:, :, ts(sq, 512)],
                                      in_=hidT[:, :, ts(sq, 512)])
                # o_w prefetch: after the phase A loads so it doesn't
                # steal DMA bandwidth from them; lands well before o-proj.
                nc.sync.dma_start(out=ow_t[:], in_=owT)
                # ACT queue: rope/attention consts (small, needed early).
                nc.scalar.dma_start(out=cos_t[:], in_=cosT)
                nc.scalar.dma_start(out=sin_t[:], in_=sinT)
                nc.scalar.dma_start(out=xt[:], in_=xT_in)
                nc.scalar.dma_start(out=tri_t[:], in_=triT)
                nc.scalar.dma_start(out=perm_t[:], in_=permT)
                for sq in range(4):
                    sqs = ds(512 * sq, 512)
                    # ---- phase A chunk: K^T then V for seq block sq ----
                    pk = [psk.tile([128, 512], F32, tag='pk', name='pk')
                          for _ in range(2)]
                    for hc in range(16):
                        for m in range(2):
                            nc.tensor.matmul(
                                pk[m][:],
                                (qkw_t[:, hc, ts(m, 128)]),
                                (hid_t[:, hc, sqs]),
                                start=(hc == 0), stop=(hc == 15))
                    # K: copy to bf16, rotate-half via PE permutation matmul,
                    # rope in place (sin sign folded on host)
                    ks = swp.tile([128, 2, 512], BF16, tag="sw")
                    for t in range(2):
                        nc.vector.tensor_copy(out=kt[:, t, sqs], in_=pk[t][:])
                    for t in range(2):
                        ksp = pss.tile([128, 512], F32, tag='sp')
                        nc.tensor.matmul(ksp[:], (perm_t[:]),
                                         (kt[:, t, sqs]),
                                         start=True, stop=True)
                        nc.vector.tensor_mul(out=ks[:, t, :], in0=ksp[:],
                                             in1=sin_t[:, sqs])
                        nc.vector.tensor_mul(out=kt[:, t, sqs],
                                             in0=kt[:, t, sqs],
                                             in1=cos_t[:, sqs])
                        nc.vector.tensor_add(out=kt[:, t, sqs],
                                             in0=kt[:, t, sqs],
                                             in1=ks[:, t, :])
                    # V: one psum group per bank slot (start=True zeroes the
                    # whole 2KB zero-region, so groups must not share a bank)
                    for st4 in range(4):
                        pvt = psv.tile([128, 256], F32, tag='pv', name='pv')
                        for hc in range(16):
                            nc.tensor.matmul(
                                pvt[:],
                                (hid_t[:, hc, ds(512 * sq + 128 * st4, 128)]),
                                (vw_t[:, hc, :]),
                                start=(hc == 0), stop=(hc == 15))
                        nc.vector.tensor_copy(
                            out=v_t[:, sq * 4 + st4, :].rearrange(
                                "p (h w) -> p h w", w=VW)[:, :, 0:HD],
                            in_=pvt[:].rearrange("p (h d) -> p h d", d=HD))

                    # ---- phase B: attention q-block j == sq, all 4 heads ----
                    j = sq
                    q0 = 512 * j
                    nk = 4 * (j + 1)
                    for h in range(HPC):
                        hp = HD * (h % 2)
                        htl = h // 2
                        pvp = pspv.tile([HD + 1, 512], F32, tag='pvp')
                        for i in range(nk):
                            r = 128 * i - q0
                            w0 = max(r, 0)
                            sp = pss.tile([128, 512], F32, tag='sp')
                            nc.tensor.matmul(
                                sp[:, ds(w0, 512 - w0)],
                                (kt[hp:hp + HD, htl, ts(i, 128)]),
                                (xt[hp:hp + HD, htl, ds(q0 + w0, 512 - w0)]),
                                start=True, stop=True)
                            pt = ppool.tile([128, 512], BF16, tag="pt")
                            nc.scalar.activation(
                                out=pt[:, ds(w0, 512 - w0)],
                                in_=sp[:, ds(w0, 512 - w0)],
                                func=mybir.ActivationFunctionType.Exp,
                                scale=0.125)
                            if r >= 0:   # diagonal: ragged triangle mask
                                nc.vector.tensor_mul(
                                    out=pt[:, ds(r, 128)], in0=pt[:, ds(r, 128)],
                                    in1=tri_t[:])
                            nc.tensor.matmul(
                                pvp[0:HD + 1, ds(w0, 512 - w0)],
                                (v_t[:, i, ds(VW * h, HD + 1)]),
                                (pt[:, ds(w0, 512 - w0)]),
                                start=(i == 0), stop=(i == nk - 1))
                        # denominator: reciprocal row, broadcast into the
                        # unused partitions 64..127 of the same psum bank
                        rec = rrp.tile([1, 512], BF16, tag="rec")
                        nc.vector.reciprocal(out=rec[:], in_=pvp[HD:HD + 1, :])
                        bc = pss.tile([HD, 512], F32, tag='sp')
                        nc.tensor.matmul(bc[:], (ones_t[:]),
                                         (rec[:]), start=True, stop=True)
                        bcs = rrp.tile([HD, 512], BF16, tag="bcs")
                        nc.vector.tensor_copy(out=bcs[:], in_=bc[:])
                        nc.vector.tensor_mul(
                            out=att_t[hp:hp + HD, htl, ds(q0, 512)],
                            in0=pvp[0:HD, :],
                            in1=bcs[:])

            if dbg:
                nc.sync.dma_start(out=kt_out[:],
                                  in_=kt[:].rearrange("p t s -> p (t s)"))
                nc.sync.dma_start(out=vt_out[:],
                                  in_=v_t[:].rearrange("p a b -> p (a b)"))
                nc.sync.dma_start(out=att_out[:],
                                  in_=att_t[:].rearrange("p t s -> p (t s)"))

            # =========== AllToAll: head-sharded -> seq-sharded ===========
            # Split by head-pair tile t: the t=0 collective fires as soon as
            # heads 0-1 finish, overlapping the t=1 attention tail and the
            # even-hc half of the o-projection with the t=1 collective.
            a2a_in = [dram.tile([NCORES, 128, SSL], BF16, name=f"a2ain{t}")
                      for t in range(2)]
            a2a_out = [dram.tile([NCORES * 128, SSL], BF16, name=f"a2aout{t}")
                       for t in range(2)]
            for t in range(2):
                nc.sync.dma_start(
                    out=a2a_in[t][:].rearrange("d p s -> p d s"),
                    in_=att_t[:, t, :].rearrange("p (d s) -> p d s", d=NCORES))
                if collective:
                    nc.gpsimd.collective_compute(
                        "AllToAll",
                        mybir.AluOpType.bypass,
                        replica_groups=[list(range(NCORES))],
                        ins=[a2a_in[t][:].opt()],
                        outs=[a2a_out[t][:].opt()],
                    )
                else:
                    # timeline-sim mock: same-size DRAM->DRAM move
                    nc.sync.dma_start(
                        out=a2a_out[t][:],
                        in_=a2a_in[t][:].rearrange("d p s -> (d p) s"))

            # =========== o-projection (sequence-parallel) ===========
            with (
                tc.tile_pool(name="af", bufs=1) as afp,
                tc.tile_pool(name="ob", bufs=1) as obp,
                tc.tile_pool(name="pso", bufs=8, space="PSUM") as pso,
            ):
                afull = afp.tile([128, 16, SSL], BF16)
                for t in range(2):
                    # a2a_out[t] rows (d p) hold global attn dims 256d+128t+p
                    # -> afull n slices t, t+2, t+4, ...
                    nc.sync.dma_start(
                        out=afull[:].rearrange(
                            "p (d u) s -> p d u s", u=2)[:, :, t, :],
                        in_=a2a_out[t][:].rearrange("(d p) s -> p d s", p=128))
                if dbg:
                    nc.sync.dma_start(out=afu_out[:],
                                      in_=afull[:].rearrange("p a b -> p (a b)"))
                osb = obp.tile([128, 2, HID], F32)
                po = [[pso.tile([128, 512], F32, tag='po', name='po')
                       for t in range(2)] for ob in range(4)]
                # even hc chunks depend only on the t=0 collective, odd on
                # t=1: run all even ones first so they overlap collective 1.
                for hc in [2 * i for i in range(8)] + [2 * i + 1 for i in range(8)]:
                    for ob in range(4):
                        for t in range(2):
                            nc.tensor.matmul(
                                po[ob][t][:],
                                (afull[:, hc, ts(t, 128)]),
                                (ow_t[:, hc, ts(ob, 512)]),
                                start=(hc == 0), stop=(hc == 15))
                for ob in range(4):
                    nc.scalar.copy(out=osb[:, 0, ts(ob, 512)],
                                   in_=po[ob][0][:])
                    nc.vector.tensor_copy(out=osb[:, 1, ts(ob, 512)],
                                          in_=po[ob][1][:])
                nc.sync.dma_start(out=out_sl[ts(0, 128), :], in_=osb[:, 0, :])
                nc.scalar.dma_start(out=out_sl[ts(1, 128), :], in_=osb[:, 1, :])

    nc.compile()
    return nc


_PROGRAM = None


def _host_inputs(hidden_states, qk_w, v_w, o_w, position_ids):
    bf16 = ml_dtypes.bfloat16
    hs = np.asarray(hidden_states, dtype=np.float32)[0]          # [S, HID]
    qk_w = np.asarray(qk_w, dtype=np.float32)
    v_w = np.asarray(v_w, dtype=np.float32)
    o_w = np.asarray(o_w, dtype=np.float32)
    pos = np.asarray(position_ids)[0].astype(np.float64)         # [S]

    hidT = np.ascontiguousarray(hs.T)                            # [HID, S]
    hidT_bf = hidT.astype(bf16)
    owT_bf = np.ascontiguousarray(o_w.T).astype(bf16)            # [HID, HID]

    inv_freq = 1.0 / (ROPE_THETA ** (np.arange(0, HD, 2, dtype=np.float64) / HD))
    freqs = pos[None, :] * inv_freq[:, None]                     # [32, S]
    emb = np.concatenate([freqs, freqs], axis=0)                 # [64, S]
    cos1 = np.cos(emb).astype(np.float32)
    sin1 = np.sin(emb).astype(np.float32)
    sin_signed = sin1.copy()
    sin_signed[:HD // 2] *= -1.0                                 # fold rotate sign
    cosT = np.tile(cos1, (2, 1)).astype(bf16)                    # [128, S]
    sinT = np.tile(sin_signed, (2, 1)).astype(bf16)

    kl = np.arange(128)[:, None]
    u = np.arange(128)[None, :]
    triT = (u >= kl).astype(bf16)                                # [128, 128]

    # rotate-half row permutation (symmetric involution, per 64-row head)
    idx = np.arange(128)
    src = (idx // HD) * HD + (idx % HD + HD // 2) % HD
    permT = np.zeros((128, 128), np.float32)
    permT[idx, src] = 1.0
    permT = permT.astype(bf16)

    in_maps = []
    for c in range(NCORES):
        rows = slice(DPC * c, DPC * (c + 1))
        xT = hidT[rows]                                          # [256, S] fp32
        # host-side X RoPE: x*cos + rotate_half(x)*sin per 64-row head
        xTs = np.empty_like(xT)
        for h in range(HPC):
            b = HD * h
            xTs[b:b + 32] = -xT[b + 32:b + HD]
            xTs[b + 32:b + HD] = xT[b:b + 32]
        cs = np.tile(cos1, (HPC, 1))                             # [256, S]
        sn = np.tile(sin1, (HPC, 1))
        xTr = (xT * cs + xTs * sn).astype(bf16)
        parts = {
            "hidT": hidT_bf,
            "qkwT": np.ascontiguousarray(qk_w[rows].T).astype(bf16),
            "vwT": np.ascontiguousarray(v_w[rows].T).astype(bf16),
            "owT": owT_bf,
            "xT": xTr,
            "cosT": cosT,
            "sinT": sinT,
            "triT": triT,
            "permT": permT,
        }
        blob = np.concatenate([parts[nm].ravel() for nm, _ in _BLOB_SPEC])
        assert blob.size == BLOB_ELEMS
        in_maps.append({"blob": blob})
    return in_maps


def kernel(hidden_states, qk_w, v_w, o_w, position_ids, **extra):
    global _PROGRAM
    if _PROGRAM is None:
        _PROGRAM = build_program()
    in_maps = _host_inputs(hidden_states, qk_w, v_w, o_w, position_ids)
    res = run_bass_kernel_spmd(_PROGRAM, in_maps, list(range(NCORES)))
    out = np.concatenate([res.results[c]["out_slice"]
                          for c in range(NCORES)], axis=0)
    return out.reshape(1, S, HID).astype(np.float32)
